# revision 1
# baseline (speedup 1.0000x reference)
"""Trainium2 Bass kernel for nn_Correlation (FlowNet-style cost volume).

Problem: input1/input2 [8, 256, 96, 128] f32 ->
         out [8, 441, 96, 128] f32
  out[b, 21*i+j, h, w] = leaky_relu_0.1( (1/256) * sum_c
        in1[b,c,h,w] * in2pad[b,c, h+2i, w+2j] )       (pad 20 each side)

Strategy (data-parallel over B across 8 cores; per core = 1 sample):
  * Displacements are even (dilation 2): pixel (h,w) only correlates with
    in2 pixels of the same (h%2, w%2) parity class. In parity space the
    dilated 21x21 patch is a dense 21x21 window.
  * Per parity class: split the 48x64 parity image into 8x16 pixel
    blocks (128 pixels = PE stationary operand). Stream the block's
    28x36 in2 parity window through the PE contracting over C=256
    (bf16, fp32 PSUM) -> band[pixel, window_col] (1008 cols, 441 useful).
  * PE operands need single-stride APs, so both inputs are rearranged
    on-chip (GPSIMD copies): in1 into parity-blocked contiguous 128-pixel
    groups; in2 into per-wb "bands" where each block's window rows are
    contiguous (36-row rolling buffer over padded parity rows).
  * Band -> SBUF (DVE) -> DRAM scratch (contiguous) -> diagonal gather
    back (per-pixel 21x21 patch; DRAM-side APs are flat so the diagonal
    is legal) -> ALIGNED[pixel, 441].
  * PE-transpose ALIGNED -> [d, pixel]; ScalarE applies
    leaky_relu(x/256) scattering into parity-interleaved row tiles;
    contiguous stores.
"""

import numpy as np

import concourse.bass as bass
import concourse.mybir as mybir
from concourse.tile import TileContext
from concourse.bass_utils import run_bass_kernel_spmd
from concourse.masks import make_identity

DT = mybir.dt

# ---- problem geometry ----
B, C, H, W = 8, 256, 96, 128
NP = 21                      # displacements per axis
ND = NP * NP                 # 441
CC = 2                       # C chunks of 128
HE, WE = H // 2, W // 2      # parity image 48 x 64
HEP, WEP = HE + 20, WE + 20  # padded parity image 68 x 84

HB, WB = 8, 16               # parity block (he, we); HB*WB = 128
WIN_H, WIN_W = HB + 20, WB + 20   # 28 x 36 window
FB = WIN_H * WIN_W           # 1008 band columns
HW = H * W                   # 12288

# in2 "bands": per (cc, hp, wp, wb) a [NSLOT, 36] contiguous-row image,
# rolling over padded parity rows (h'e in [0, 68), slot = h'e % NSLOT).
# NSLOT=40 (not 36) so a prefetched group only overwrites rows of
# ALREADY-FINISHED block-rows (dependency slack for pipelining).
NSLOT = 40
BAND_PITCH = NSLOT * WIN_W               # 1296
N_BANDS = CC * 2 * 2 * 4                 # 32
BANDS_F = N_BANDS * BAND_PITCH           # 41472

IN1BLK_F = 2 * 2 * CC * 4 * 128          # 4096 per block-row slab
STG_F = CC * 16 * W                      # 4096 (16 full-res rows)

_MAX_WAITS = 1


def _split_excess_waits(nc):
    """This walrus build accepts only ONE sync-wait per instruction; Tile
    emits multi-waits. Hoist excess waits onto same-engine NOPs inserted
    right before the over-subscribed instruction."""
    nid = 0
    for f in nc.m.functions:
        for blk in f.blocks:
            insts = list(blk.instructions)
            out = []
            changed = False
            for inst in insts:
                si = inst.sync_info
                if si is not None and si.on_wait and len(si.on_wait) > _MAX_WAITS:
                    waits = list(si.on_wait)
                    extra, keep = waits[:-_MAX_WAITS], waits[-_MAX_WAITS:]
                    for k in range(0, len(extra), _MAX_WAITS):
                        nop = mybir.InstNoOp(name=f"I-waitsplit-{nid}", ins=[], outs=[])
                        nid += 1
                        nop.engine = inst.engine
                        nop.sync_info = mybir.SyncInfo(
                            on_wait=extra[k : k + _MAX_WAITS], on_update=[]
                        )
                        out.append(nop)
                        changed = True
                    si.on_wait = keep
                    inst.sync_info = si
                out.append(inst)
            if changed:
                blk.instructions = out
    return nc


def _ap(t, off_extra, dims):
    return bass.AP(tensor=t.tensor, offset=t.offset + off_extra, ap=dims)


def _band_base(cc, hp, wp, wb):
    return (((cc * 2 + hp) * 2 + wp) * 4 + wb) * BAND_PITCH


def _slot_runs(lo, hi):
    """Contiguous (slot, h'e, count) runs for padded parity rows [lo, hi)."""
    runs = []
    r = lo
    while r < hi:
        s = r % NSLOT
        n = min(hi - r, NSLOT - s)
        runs.append((s, r, n))
        r += n
    return runs


def _row_pieces(a):
    """Matmul N-pieces for block-row a: [(i0, ni), ...] respecting the
    rolling-slot wrap and the 504-column PSUM bank split."""
    s0 = a % NSLOT
    w = NSLOT - s0
    runs = [(0, 28)] if w >= 28 else [(0, w), (w, 28 - w)]
    pieces = []
    for i0, n in runs:
        end = i0 + n
        for b0, b1 in ((0, 14), (14, 28)):
            lo, hi = max(i0, b0), min(end, b1)
            if lo < hi:
                pieces.append((lo, hi - lo))
    return pieces


def _build_nc(debug=False, waitsplit=True):
    nc = bass.Bass()
    in1_d = nc.dram_tensor("in1", [C, H, W], DT.float32, kind="ExternalInput")
    in2_d = nc.dram_tensor("in2", [C, H, W], DT.float32, kind="ExternalInput")
    out_d = nc.dram_tensor("out", [ND, H, W], DT.float32, kind="ExternalOutput")
    dbg = {}
    if debug:
        dbg["bands"] = nc.dram_tensor(
            "dbg_bands", [128, BANDS_F], DT.bfloat16, kind="ExternalOutput"
        )
        dbg["in1blk"] = nc.dram_tensor(
            "dbg_in1blk", [128, IN1BLK_F], DT.bfloat16, kind="ExternalOutput"
        )
        dbg["band_sb"] = nc.dram_tensor(
            "dbg_band_sb", [128, FB], DT.float32, kind="ExternalOutput"
        )
        dbg["alig"] = nc.dram_tensor(
            "dbg_alig", [128, ND], DT.float32, kind="ExternalOutput"
        )
        dbg["tr"] = nc.dram_tensor(
            "dbg_tr", [128, 512], DT.float32, kind="ExternalOutput"
        )
        dbg["bands2"] = nc.dram_tensor(
            "dbg_bands2", [128, BANDS_F], DT.bfloat16, kind="ExternalOutput"
        )
        dbg["band_sb2"] = nc.dram_tensor(
            "dbg_band_sb2", [128, FB], DT.float32, kind="ExternalOutput"
        )
        dbg["alig2"] = nc.dram_tensor(
            "dbg_alig2", [128, ND], DT.float32, kind="ExternalOutput"
        )

    with TileContext(nc) as tc:
        with (
            tc.tile_pool(name="constp", bufs=1) as constp,
            tc.tile_pool(name="bandsp", bufs=1) as bandsp,
            tc.tile_pool(name="stgp", bufs=2) as stgp,
            tc.tile_pool(name="in1p", bufs=2) as in1p,
            tc.tile_pool(name="bandsbp", bufs=2) as bandsbp,
            tc.tile_pool(name="aligp", bufs=3) as aligp,
            tc.tile_pool(name="outp", bufs=1) as outp,
            tc.tile_pool(name="relup", bufs=2) as relup,
            tc.tile_pool(name="psp", bufs=2, space="PSUM") as psp,
            tc.tile_pool(name="trpp", bufs=2, space="PSUM") as trpp,
            tc.tile_pool(name="dramp", bufs=4, space="DRAM") as dramp,
        ):
            identity = constp.tile([128, 128], DT.float32)
            make_identity(nc, identity)

            bands = constp.tile([128, BANDS_F], DT.bfloat16)

            # zero everything once: covers w-padding columns and all
            # initial padding rows (contiguous write = precise dep tracking)
            nc.vector.memset(bands[:, :], 0.0)

            def build_group(g):
                """Fill band rows for padded parity rows [8g, 8g+8)."""
                glo, ghi = 8 * g, min(8 * g + 8, HEP)
                # zero spans (padding rows) - skip for g<2 (initial memset
                # covered them); needed when slots are being recycled
                for zlo, zhi in ((glo, min(ghi, 10)), (max(glo, 58), ghi)):
                    if zlo >= zhi or zhi <= NSLOT:
                        continue
                    for s0, _, n in _slot_runs(zlo, zhi):
                        for cc in range(CC):
                            for hp in range(2):
                                for wp in range(2):
                                    for wb in range(4):
                                        nc.vector.memset(
                                            _ap(
                                                bands,
                                                _band_base(cc, hp, wp, wb)
                                                + s0 * WIN_W,
                                                [[BANDS_F, 128], [1, n * WIN_W]],
                                            ),
                                            0.0,
                                        )
                # data span
                dlo, dhi = max(glo, 10), min(ghi, 58)
                if dlo >= dhi:
                    return
                h0, nh = 2 * (dlo - 10), 2 * (dhi - dlo)
                stg = stgp.tile([128, STG_F], DT.bfloat16, name="stg", bufs=1)
                for cc in range(CC):
                    nc.gpsimd.dma_start(
                        _ap(stg, cc * 16 * W, [[STG_F, 128], [1, nh * W]]),
                        in2_d[cc * 128 : (cc + 1) * 128, h0 : h0 + nh, :],
                    )
                for cc in range(CC):
                    for hp in range(2):
                        for wp in range(2):
                            for s0, he0, n in _slot_runs(dlo, dhi):
                                src_r = 2 * (he0 - dlo) + hp
                                for wb in range(4):
                                    # valid u range for this wb (w-padding)
                                    u0 = 10 if wb == 0 else 0
                                    u1 = 26 if wb == 3 else WIN_W
                                    nu = u1 - u0
                                    s_ap = _ap(
                                        stg,
                                        cc * 16 * W
                                        + src_r * W
                                        + (2 * (16 * wb + u0 - 10) + wp),
                                        [[STG_F, 128], [2 * W, n], [2, nu]],
                                    )
                                    d_ap = _ap(
                                        bands,
                                        _band_base(cc, hp, wp, wb)
                                        + s0 * WIN_W
                                        + u0,
                                        [[BANDS_F, 128], [WIN_W, n], [1, nu]],
                                    )
                                    nc.gpsimd.tensor_copy(d_ap, s_ap)

            def build_in1_slab(ai):
                """Parity-blocked in1 for block-row ai -> [128, IN1BLK_F]."""
                stg1 = stgp.tile([128, STG_F], DT.bfloat16, name="stg1", bufs=1)
                for cc in range(CC):
                    nc.gpsimd.dma_start(
                        _ap(stg1, cc * 16 * W, [[STG_F, 128], [1, 16 * W]]),
                        in1_d[cc * 128 : (cc + 1) * 128, 16 * ai : 16 * ai + 16, :],
                    )
                blk = in1p.tile([128, IN1BLK_F], DT.bfloat16, name="in1blk")
                for cc in range(CC):
                    for hp in range(2):
                        for wp in range(2):
                            src = _ap(
                                stg1,
                                cc * 16 * W + hp * W + wp,
                                [[STG_F, 128], [32, 4], [2 * W, HB], [2, WB]],
                            )
                            dst = _ap(
                                blk,
                                ((cc * 2 + hp) * 2 + wp) * 512,
                                [[IN1BLK_F, 128], [128, 4], [16, HB], [1, WB]],
                            )
                            nc.gpsimd.tensor_copy(dst, src)
                return blk

            # prologue: band groups 0-3, first in1 slab
            for g in range(4):
                build_group(g)
            in1blk = build_in1_slab(0)
            if debug:
                nc.sync.dma_start(
                    bass.AP(tensor=dbg["bands"], offset=0, ap=[[BANDS_F, 128], [1, BANDS_F]]),
                    bands[:, :],
                )
                nc.sync.dma_start(
                    bass.AP(tensor=dbg["in1blk"], offset=0, ap=[[IN1BLK_F, 128], [1, IN1BLK_F]]),
                    in1blk[:, :],
                )

            for k, a in enumerate(range(0, HE, HB)):  # 6 block-rows
                out_t = [
                    outp.tile([128, 16 * W], DT.float32, name=f"outt{dc}")
                    for dc in range(4)
                ]
                pieces = _row_pieces(a)
                for hp in range(2):
                    for wp in range(2):
                        for wb in range(4):
                            ps_pieces = [
                                psp.tile([128, 504], DT.float32, name="ps_a"),
                                psp.tile([128, 504], DT.float32, name="ps_b"),
                            ]
                            # one PSUM accumulation group per bank: start
                            # only on the bank's first write, stop on its last
                            bank_pieces = {0: [], 1: []}
                            for i0, ni in pieces:
                                bank_pieces[0 if i0 < 14 else 1].append((i0, ni))
                            for cc in range(CC):
                                lhsT = _ap(
                                    in1blk,
                                    (((cc * 2 + hp) * 2 + wp) * 4 + wb) * 128,
                                    [[IN1BLK_F, 128], [1, 128]],
                                )
                                for i0, ni in pieces:
                                    s0 = (a + i0) % NSLOT
                                    rhs = _ap(
                                        bands,
                                        _band_base(cc, hp, wp, wb) + s0 * WIN_W,
                                        [[BANDS_F, 128], [1, ni * WIN_W]],
                                    )
                                    pi = 0 if i0 < 14 else 1
                                    c0 = (i0 - 14 * pi) * WIN_W
                                    bp = bank_pieces[pi]
                                    nc.tensor.matmul(
                                        ps_pieces[pi][:, c0 : c0 + ni * WIN_W],
                                        lhsT,
                                        rhs,
                                        start=(cc == 0 and (i0, ni) == bp[0]),
                                        stop=(cc == CC - 1 and (i0, ni) == bp[-1]),
                                    )
                            # band -> SBUF -> DRAM
                            band_sb = bandsbp.tile([128, FB], DT.float32, name="band_sb")
                            nc.vector.tensor_copy(band_sb[:, 0:504], ps_pieces[0][:, :])
                            nc.vector.tensor_copy(
                                band_sb[:, 504:1008], ps_pieces[1][:, :]
                            )
                            if debug and (a, hp, wp, wb) == (0, 0, 0, 0):
                                nc.sync.dma_start(
                                    bass.AP(tensor=dbg["band_sb"], offset=0, ap=[[FB, 128], [1, FB]]),
                                    band_sb[:, :],
                                )
                            if debug and (a, hp, wp, wb) == (16, 0, 0, 0):
                                nc.sync.dma_start(
                                    bass.AP(tensor=dbg["band_sb2"], offset=0, ap=[[FB, 128], [1, FB]]),
                                    band_sb[:, :],
                                )
                            bdram = dramp.tile([128, FB], DT.float32, name="bdram")
                            nc.sync.dma_start(bdram[:, :], band_sb[:, :])
                            # diagonal gather DRAM -> ALIGNED[pixel, 441]
                            alig = aligp.tile([128, ND], DT.float32, name="alig")
                            for he in range(HB):
                                src = _ap(
                                    bdram,
                                    he * (16 * FB + WIN_W),
                                    [[FB + 1, 16], [WIN_W, NP], [1, NP]],
                                )
                                dst = _ap(
                                    alig,
                                    he * 16 * ND,
                                    [[ND, 16], [NP, NP], [1, NP]],
                                )
                                eng = nc.sync if he % 2 == 0 else nc.scalar
                                eng.dma_start(dst, src)
                            if debug and (a, hp, wp, wb) == (0, 0, 0, 0):
                                nc.sync.dma_start(
                                    bass.AP(tensor=dbg["alig"], offset=0, ap=[[ND, 128], [1, ND]]),
                                    alig[:, :],
                                )
                            if debug and (a, hp, wp, wb) == (16, 0, 0, 0):
                                nc.sync.dma_start(
                                    bass.AP(tensor=dbg["alig2"], offset=0, ap=[[ND, 128], [1, ND]]),
                                    alig[:, :],
                                )
                            # transpose pixel-major -> d-major
                            tr = trpp.tile([128, 512], DT.float32, name="tr")
                            for dc in range(4):
                                dlo = dc * 128
                                nd = min(128, ND - dlo)
                                nc.tensor.transpose(
                                    tr[0:nd, dc * 128 : dc * 128 + 128],
                                    alig[:, dlo : dlo + nd],
                                    identity[:, :],
                                )
                            if debug and (a, hp, wp, wb) == (0, 0, 0, 0):
                                nc.vector.tensor_copy(band_sb[:, 0:512], tr[:, :])
                                nc.sync.dma_start(
                                    bass.AP(tensor=dbg["tr"], offset=0, ap=[[512, 128], [1, 512]]),
                                    band_sb[:, 0:512],
                                )
                            # epilogue: leaky(x/C) = 0.1*x/C + relu(0.9*x/C)
                            relu_sb = relup.tile([128, 512], DT.float32, name="relu_sb")
                            for dc in range(4):
                                dlo = dc * 128
                                nd = min(128, ND - dlo)
                                nc.scalar.activation(
                                    relu_sb[0:nd, dc * 128 : dc * 128 + 128],
                                    tr[0:nd, dc * 128 : dc * 128 + 128],
                                    mybir.ActivationFunctionType.Relu,
                                    bias=0.0,
                                    scale=0.9 / C,
                                )
                            for dc in range(4):
                                dlo = dc * 128
                                nd = min(128, ND - dlo)
                                t_ap = _ap(
                                    tr, dc * 128, [[512, nd], [16, HB], [1, WB]]
                                )
                                r_ap = _ap(
                                    relu_sb, dc * 128, [[512, nd], [16, HB], [1, WB]]
                                )
                                dst = _ap(
                                    out_t[dc],
                                    hp * W + 32 * wb + wp,
                                    [[16 * W, nd], [2 * W, HB], [2, WB]],
                                )
                                nc.vector.scalar_tensor_tensor(
                                    dst,
                                    t_ap,
                                    0.1 / C,
                                    r_ap,
                                    mybir.AluOpType.mult,
                                    mybir.AluOpType.add,
                                )
                # prefetch next band group / in1 slab
                if 4 + k < 9:
                    build_group(4 + k)
                if k + 1 < 6:
                    in1blk = build_in1_slab(k + 1)
                # stores for this block-row
                for dc in range(4):
                    dlo = dc * 128
                    nd = min(128, ND - dlo)
                    dst = bass.AP(
                        tensor=out_d,
                        offset=dlo * HW + (2 * a) * W,
                        ap=[[HW, nd], [1, 16 * W]],
                    )
                    nc.sync.dma_start(dst, out_t[dc][0:nd, :])
            if debug:
                nc.sync.dma_start(
                    bass.AP(tensor=dbg["bands2"], offset=0, ap=[[BANDS_F, 128], [1, BANDS_F]]),
                    bands[:, :],
                )

    if waitsplit:
        _split_excess_waits(nc)
    return nc


_NC_CACHE = None


def _get_nc():
    global _NC_CACHE
    if _NC_CACHE is None:
        _NC_CACHE = _build_nc()
    return _NC_CACHE


def kernel(input1, input2):
    input1 = np.ascontiguousarray(np.asarray(input1, dtype=np.float32))
    input2 = np.ascontiguousarray(np.asarray(input2, dtype=np.float32))
    assert input1.shape == (B, C, H, W) and input2.shape == (B, C, H, W)
    nc = _get_nc()
    in_maps = [{"in1": input1[b], "in2": input2[b]} for b in range(B)]
    res = run_bass_kernel_spmd(nc, in_maps, core_ids=list(range(B)))
    return np.stack([res.results[b]["out"] for b in range(B)], axis=0)

